# revision 1
# baseline (speedup 1.0000x reference)
"""Trainium2 Bass kernel for nn_Attention_3d (B=1, C=64, D=48, W=128, H=128, 8 heads).

Sharding: depth D split across 8 cores (6 slices each), weights replicated.

Per-core pipeline (per depth slice d):
  1. QKV conv as a "transposed conv": stationary = x w-pair chunks
     [c-pair(128) x h(128)] (host interleaves w-even/w-odd channels on
     partitions 0-63/64-127), moving = blockdiag([wq|wk|wv]^T) [128,384],
     fp32r -> PSUM [h, 384] per w-pair -> evac:
       - Q,K (+bias) via DVE tensor_add -> qkbuf [h, qk, c, w] fp32r
       - V -> vT tiles [h, c, w] bf16 (ScalarE copy)
  2. V tiles PE-transposed -> v_nat [w, c, 129] fp32r (col 128 = ones).
  3. Per channel pair: scoresT = kT^T @ [qT_c|qT_c'] (fp32r, N=256,
     half-garbage), exp(x - 80) on ScalarE (shift cancels in softmax,
     avoids fp32 overflow) -> ET [v,w] fp32r,
     AV paired: o = ET^T @ [v_c|1|v_c'|1] fp32r -> PSUM [w, 258] (col 128 =
     softmax denominator), reciprocal + per-partition scale on evac ->
     obuf [w, h, c] fp32r.
  4. O regroup: PE-transpose [w, (h-pair, c)] blocks -> [c-pair, w] chunks,
     final conv with blockdiag(wo^T) fp32r, bias bo' = bo + wo@bv (v-bias
     folded analytically), -> Y groups DMA'd in a device-friendly layout,
     decoded on host.
"""
import time
import numpy as np
import ml_dtypes
from contextlib import ExitStack

B, C, D, W, H = 1, 64, 48, 128, 128
NCORES = 8
DLOC = D // NCORES  # 6
QSCALE = float(8.0 ** -0.25)

_f32 = np.float32
_bf16 = ml_dtypes.bfloat16
DEBUG = False


def round_fp32r(x):
    u = np.ascontiguousarray(x, dtype=np.float32).view(np.uint32)
    lsb = (u >> 12) & 1
    u = (u + 0x7FF + lsb) & np.uint32(0xFFFFF000)
    return u.view(np.float32)


def _build_bass():
    import concourse.bacc as bacc
    import concourse.bass as bass
    import concourse.mybir as mybir
    import concourse.tile as tile

    fr = mybir.dt.float32r
    f32 = mybir.dt.float32
    bf = mybir.dt.bfloat16
    fh = mybir.dt.float16
    AF = mybir.ActivationFunctionType

    nc = bacc.Bacc("TRN2", target_bir_lowering=False, debug=False)

    xp_d = nc.dram_tensor("xp", [128, DLOC * 64 * 128], fr, kind="ExternalInput").ap()
    wbd_d = nc.dram_tensor("wbd", [128, 384], fr, kind="ExternalInput").ap()
    bqk_d = nc.dram_tensor("bqk", [128, 2 * 2 * 128], f32, kind="ExternalInput").ap()
    ident_d = nc.dram_tensor("ident", [128, 128], bf, kind="ExternalInput").ap()
    wobd_d = nc.dram_tensor("wobd", [128, 128], fr, kind="ExternalInput").ap()
    identr_d = nc.dram_tensor("identr", [128, 128], fr, kind="ExternalInput").ap()
    bo2_d = nc.dram_tensor("bo2", [128, 1], f32, kind="ExternalInput").ap()
    y_d = nc.dram_tensor("y", [DLOC, 16, 128, 512], f32, kind="ExternalOutput").ap()
    if DEBUG:
        dbg_qk = nc.dram_tensor("dbg_qk", [128, 2 * 64 * 128], f32, kind="ExternalOutput").ap()
        dbg_vb = nc.dram_tensor("dbg_vb", [128, 64 * 129], bf, kind="ExternalOutput").ap()
        dbg_ob = nc.dram_tensor("dbg_ob", [128, 128 * 64], bf, kind="ExternalOutput").ap()
        dbg_sc = nc.dram_tensor("dbg_sc", [128, 512], f32, kind="ExternalOutput").ap()

    with tile.TileContext(nc) as tc, ExitStack() as ctx:
        const = ctx.enter_context(tc.tile_pool(name="const", bufs=1))
        xpool = ctx.enter_context(tc.tile_pool(name="xpool", bufs=1))
        qkpool = ctx.enter_context(tc.tile_pool(name="qkpool", bufs=1))
        vtpool = ctx.enter_context(tc.tile_pool(name="vtpool", bufs=1))
        vbpool = ctx.enter_context(tc.tile_pool(name="vbpool", bufs=1))
        etpool = ctx.enter_context(tc.tile_pool(name="etpool", bufs=3))
        rcpool = ctx.enter_context(tc.tile_pool(name="rcpool", bufs=6))
        obpool = ctx.enter_context(tc.tile_pool(name="obpool", bufs=1))
        ochpool = ctx.enter_context(tc.tile_pool(name="ochpool", bufs=3))
        ypool = ctx.enter_context(tc.tile_pool(name="ypool", bufs=3))

        pconv = ctx.enter_context(tc.tile_pool(name="pconv", bufs=2, space="PSUM"))
        pscp = ctx.enter_context(tc.tile_pool(name="pscp", bufs=2, space="PSUM"))
        pavp = ctx.enter_context(tc.tile_pool(name="pavp", bufs=1, space="PSUM"))
        ptrp = ctx.enter_context(tc.tile_pool(name="ptrp", bufs=1, space="PSUM"))
        pyp = ctx.enter_context(tc.tile_pool(name="pyp", bufs=1, space="PSUM"))

        # constants
        wbd_sb = const.tile([128, 384], fr)
        nc.gpsimd.dma_start(wbd_sb[:], wbd_d[:])
        bqk_sb = const.tile([128, 2, 2, 128], f32)
        nc.gpsimd.dma_start(
            bqk_sb[:], bqk_d[:].rearrange("p (a b c) -> p a b c", a=2, b=2)
        )
        ident_sb = const.tile([128, 128], bf)
        nc.gpsimd.dma_start(ident_sb[:], ident_d[:])
        wobd_sb = const.tile([128, 128], fr)
        nc.gpsimd.dma_start(wobd_sb[:], wobd_d[:])
        identr_sb = const.tile([128, 128], fr)
        nc.gpsimd.dma_start(identr_sb[:], identr_d[:])
        eshift_sb = const.tile([128, 1], f32)
        nc.gpsimd.memset(eshift_sb[:], -80.0)
        bo2_sb = const.tile([128, 1], f32)
        nc.gpsimd.dma_start(bo2_sb[:], bo2_d[:])
        ones_bf = const.tile([128, 64], bf)
        nc.gpsimd.memset(ones_bf[:], 1.0)


        for d in range(DLOC):
            xp_sb = xpool.tile([128, 64 * 128], fr)
            nc.gpsimd.dma_start(
                xp_sb[:], xp_d[:, d * 8192:(d + 1) * 8192]
            )

            qkbuf = qkpool.tile([128, 2, 64, 128], fr)   # [h, qk, c, w]
            vtbuf = vtpool.tile([128, 64, 128], bf)      # [h, c, w]

            # ---- stage 1: QKV conv sweep (w-pairs, 2 pairs per psum tile)
            for wp in range(64):
                pc = pconv.tile([128, 512], f32)
                nc.tensor.matmul(
                    pc[:, 0:384],
                    xp_sb[:, wp * 128:(wp + 1) * 128],
                    wbd_sb[:],
                    start=True, stop=True,
                )
                src = pc[:, 0:384].rearrange("p (w2 c) -> p w2 c", w2=2)
                # Q,K part (+bias), fp32r out
                dst_qk = qkbuf[:, :, :, 2 * wp:2 * wp + 2].rearrange(
                    "p qk c w2 -> p w2 (qk c)"
                )
                nc.vector.tensor_add(dst_qk, src[:, :, 0:128], bqk_sb[:, 0])
                # V part, bf16 out
                dst_v = vtbuf[:, :, 2 * wp:2 * wp + 2].rearrange(
                    "p c w2 -> p w2 c"
                )
                nc.scalar.copy(dst_v, src[:, :, 128:192])

            # ---- stage 2: V transposes -> v_nat
            vbuf = vbpool.tile([128, 64, 129], fr)       # [w(v), c, h+ones]
            nc.vector.tensor_copy(vbuf[:, :, 128], ones_bf[:])
            for c4 in range(0, 64, 4):
                pt = ptrp.tile([128, 512], bf, tag="pt")
                for j in range(4):
                    nc.tensor.transpose(
                        pt[:, j * 128:(j + 1) * 128],
                        vtbuf[:, c4 + j, :], ident_sb[:],
                    )
                nc.vector.tensor_copy(
                    vbuf[:, c4:c4 + 4, 0:128],
                    pt[:].rearrange("p (a b) -> p a b", a=4),
                )

            # ---- stage 3: attention per channel pair
            obuf = obpool.tile([128, 128, 64], fr)       # [w, h, c]
            for c2 in range(0, 64, 2):
                psc = pscp.tile([128, 2, 256], f32)      # one bank
                flat = psc[:].rearrange("p a b -> p (a b)")
                rhs_pair = qkbuf[:, 0, c2:c2 + 2, :].rearrange("p c w -> p (c w)")
                nc.tensor.matmul(
                    psc[:, 0, 0:256], qkbuf[:, 1, c2, :], rhs_pair,
                    start=True, stop=True,
                )
                nc.tensor.matmul(
                    flat[:, 128:384], qkbuf[:, 1, c2 + 1, :], rhs_pair,
                    start=True, stop=True,
                )
                if DEBUG and d == 0 and c2 == 0:
                    scf = ypool.tile([128, 512], f32, tag="dbgs")
                    nc.vector.tensor_copy(scf[:], psc[:].rearrange("p a b -> p (a b)"))
                    nc.gpsimd.dma_start(dbg_sc[:], scf[:])
                et = etpool.tile([128, 2, 128], fr)
                nc.scalar.activation(et[:], psc[:, :, 0:128], AF.Exp,
                                     bias=eshift_sb[:])

                pav = pavp.tile([128, 2, 512], f32)
                rhs_av = vbuf[:, c2:c2 + 2, :].rearrange("p c h -> p (c h)")
                for j in (0, 1):
                    nc.tensor.matmul(
                        pav[:, j, 0:258], et[:, j, :], rhs_av,
                        start=True, stop=True,
                    )
                rc = rcpool.tile([128, 2], f32)
                nc.scalar.copy(rc[:], pav[:, :, 128])
                rr = rcpool.tile([128, 2], f32, tag="rr")
                nc.vector.reciprocal(rr[:], rc[:])
                nc.vector.tensor_scalar_mul(
                    obuf[:, :, c2], pav[:, 0, 0:128], rr[:, 0:1]
                )
                nc.vector.tensor_scalar_mul(
                    obuf[:, :, c2 + 1], pav[:, 1, 129:257], rr[:, 1:2]
                )

            if DEBUG and d == 0:
                qkf = qkbuf[:].rearrange("p a c w -> p (a c w)").bitcast(f32)
                nc.gpsimd.dma_start(dbg_qk[:], qkf)
                nc.gpsimd.dma_start(dbg_vb[:], vbuf[:].rearrange("p a c -> p (a c)"))
                nc.gpsimd.dma_start(dbg_ob[:], obuf[:].rearrange("p a c -> p (a c)"))

            # ---- stage 4: O regroup + final conv
            for g in range(16):
                pt = ptrp.tile([128, 512], fr, tag="pt")
                for t in range(4):
                    hp = g * 4 + t
                    nc.tensor.transpose(
                        pt[:, t * 128:(t + 1) * 128],
                        obuf[:, 2 * hp:2 * hp + 2, :].rearrange("p a c -> p (a c)"),
                        identr_sb[:],
                    )
                och = ochpool.tile([128, 512], fr)
                nc.vector.tensor_copy(och[:], pt[:])
                py = pyp.tile([128, 512], f32)
                nc.tensor.matmul(py[:], wobd_sb[:], och[:], start=True, stop=True)
                yb = ypool.tile([128, 512], f32)
                nc.scalar.activation(
                    yb[:], py[:], AF.Identity, bias=bo2_sb[:]
                )
                nc.gpsimd.dma_start(y_d[d, g], yb[:])

    nc.compile()
    return nc


def _prep_inputs(x, wq, bq, wk, bk, wv, bv, wo, bo):
    """Build per-core input maps."""
    x = np.asarray(x, _f32)[0]           # [64, 48, 128, 128]
    wq2 = np.asarray(wq, _f32) * QSCALE
    bq2 = np.asarray(bq, _f32) * QSCALE
    wk = np.asarray(wk, _f32); bk = np.asarray(bk, _f32)
    wv = np.asarray(wv, _f32); bv = np.asarray(bv, _f32)
    wo = np.asarray(wo, _f32); bo = np.asarray(bo, _f32)

    A = np.zeros((64, 192), _f32)
    A[:, 0:64] = wq2.T
    A[:, 64:128] = wk.T
    A[:, 128:192] = wv.T
    wbd = np.zeros((128, 384), _f32)
    wbd[0:64, 0:192] = A
    wbd[64:128, 192:384] = A
    wbd = round_fp32r(wbd)

    bqk_row = np.concatenate([bq2, bk])            # [128]
    bqk = np.ascontiguousarray(
        np.broadcast_to(bqk_row[None, None, :], (128, 4, 128)).reshape(128, 512),
        dtype=_f32,
    )

    ident = np.eye(128, dtype=_f32).astype(_bf16)

    wobd = np.zeros((128, 128), _f32)
    wobd[0:64, 0:64] = wo.T
    wobd[64:128, 64:128] = wo.T
    wobd = round_fp32r(wobd)
    identr = round_fp32r(np.eye(128, dtype=_f32))

    bo2v = (bo + wo @ bv).astype(_f32)
    bo2 = np.concatenate([bo2v, bo2v]).reshape(128, 1)

    in_maps = []
    for i in range(NCORES):
        xc = x[:, i * DLOC:(i + 1) * DLOC]          # [64, 6, 128, 128]
        xp = np.empty((128, DLOC, 64, 128), _f32)
        xp[0:64] = xc[:, :, 0::2, :]                # w even
        xp[64:128] = xc[:, :, 1::2, :]              # w odd
        xp = round_fp32r(xp.reshape(128, DLOC * 64 * 128))
        in_maps.append({
            "xp": xp, "wbd": wbd, "bqk": bqk, "ident": ident,
            "identr": identr, "wobd": wobd, "bo2": bo2,
        })
    return in_maps


def _decode_outputs(results):
    """results: list of per-core dicts with 'y' [6, 16, 128, 512]."""
    outs = []
    for r in results:
        y = np.asarray(r["y"], _f32).reshape(DLOC, 16, 2, 64, 4, 128)
        # dims: [d, g, ph, c, t, w] ; h = g*8 + t*2 + ph
        Y = np.transpose(y, (3, 0, 5, 1, 4, 2)).reshape(64, DLOC, 128, 128)
        outs.append(Y)
    return np.concatenate(outs, axis=1)[None]        # [1, 64, 48, 128, 128]


_CACHE = {}


def _get_runner():
    """Build bass program + cached jitted pjrt callable."""
    if "runner" in _CACHE:
        return _CACHE["runner"]
    import jax
    import jax.numpy as jnp
    from jax.sharding import Mesh, PartitionSpec
    from jax.experimental.shard_map import shard_map
    import concourse.mybir as mybir
    from concourse import bass2jax
    from concourse.bass2jax import _bass_exec_p, install_neuronx_cc_hook

    install_neuronx_cc_hook()
    nc = _build_bass()

    partition_name = (
        nc.partition_id_tensor.name if nc.partition_id_tensor else None
    )
    in_names, out_names, out_avals = [], [], []
    for alloc in nc.m.functions[0].allocations:
        if not isinstance(alloc, mybir.MemoryLocationSet):
            continue
        name = alloc.memorylocations[0].name
        if alloc.kind == "ExternalInput":
            if name != partition_name:
                in_names.append(name)
        elif alloc.kind == "ExternalOutput":
            out_names.append(name)
            out_avals.append(
                jax.core.ShapedArray(
                    tuple(alloc.tensor_shape), mybir.dt.np(alloc.dtype)
                )
            )
    n_params = len(in_names)
    zero_shapes = [(a.shape, a.dtype) for a in out_avals]
    all_in_names = list(in_names) + list(out_names)
    if partition_name is not None:
        all_in_names.append(partition_name)

    def _body(*args):
        operands = list(args)
        if partition_name is not None:
            operands.append(bass2jax.partition_id_tensor())
        outs = _bass_exec_p.bind(
            *operands,
            out_avals=tuple(out_avals),
            in_names=tuple(all_in_names),
            out_names=tuple(out_names),
            lowering_input_output_aliases=(),
            sim_require_finite=True,
            sim_require_nnan=True,
            nc=nc,
        )
        return tuple(outs)

    devices = jax.devices()[:NCORES]
    mesh = Mesh(np.asarray(devices), ("core",))
    n_outs = len(out_names)
    in_specs = (PartitionSpec("core"),) * (n_params + n_outs)
    out_specs = (PartitionSpec("core"),) * n_outs
    donate = tuple(range(n_params, n_params + n_outs))
    sharded = jax.jit(
        shard_map(_body, mesh=mesh, in_specs=in_specs, out_specs=out_specs,
                  check_rep=False),
        donate_argnums=donate,
        keep_unused=True,
    )

    def run(in_maps):
        concat_in = [
            np.concatenate([np.asarray(in_maps[c][nm]) for c in range(NCORES)],
                           axis=0)
            for nm in in_names
        ]
        concat_zeros = [
            np.zeros((NCORES * s[0],) + tuple(s[1:]), dt)
            for (s, dt) in zero_shapes
        ]
        out = sharded(*concat_in, *concat_zeros)
        res = []
        for c in range(NCORES):
            res.append({
                nm: np.asarray(out[i]).reshape(NCORES, *zero_shapes[i][0])[c]
                for i, nm in enumerate(out_names)
            })
        return res, (sharded, in_names, zero_shapes, out_names)

    _CACHE["runner"] = run
    return run


def kernel(**inputs):
    run = _get_runner()
    in_maps = _prep_inputs(**inputs)
    results, _ = run(in_maps)
    return _decode_outputs(results)


if __name__ == "__main__":
    import reference
    t0 = time.time()
    ins = {k: np.asarray(v) for k, v in reference.setup_inputs().items()}
    exp = np.asarray(reference.reference(**ins))
    t1 = time.time()
    print(f"reference: {t1 - t0:.1f}s", flush=True)
    act = kernel(**ins)
    t2 = time.time()
    print(f"kernel: {t2 - t1:.1f}s", flush=True)
    err = np.abs(act - exp)
    scale = np.abs(exp).mean()
    print(f"abs err max={err.max():.3e} mean={err.mean():.3e} "
          f"rel(max/scale)={err.max() / scale:.3e} "
          f"rel_mean={(err / (np.abs(exp) + 1e-6)).mean():.3e}", flush=True)

